# revision 22
# baseline (speedup 1.0000x reference)
"""Distributed Trainium2 kernel for relative-position causal attention.

N=M=2048, B=1, D=1024, H=16, DQK=DV=64, OFFSET=0.
2 heads per core on 8 NeuronCores. Per core:
  - chunk-major input loads (one 3D DMA per 512-col chunk of xq/xkv/sc)
    interleaved with projections so attention starts early and the PE
    stays continuously busy (p-state ramp)
  - per 128-row block: pos logits Q PErev^T in fp16 (psum copy, narrowed
    to the needed rel-position window), diagonal skew-gather via
    SBUF->SBUF DMA (row stride F2-1), content Q K^T added on DVE,
    exp (unnormalized) -> bf16 P
  - ONE xbar dma_start_transpose per (block, head) converts P [128, span]
    into tile-major P^T slots; ctx = V^T P^T as wide [128c, 65, 512]
    matmuls where V carries a ones column so psum row 64 = softmax denom;
    normalize via K=1 matmul broadcast of the denom row + DVE divide
  - AllToAll split: n-tiles 0-7 ship mid-flight (gpsimd reaches the
    collective only after block 13's skews so nothing queues behind it;
    blocks 14/15 issue their skews from the scalar HW queue), tiles 8-15
    at the end; each core out-projects rows {128c, 1024+128c}
"""

import sys

sys.path.insert(0, "/opt/trn_rl_repo")

import numpy as np
import ml_dtypes

from concourse import bass, bacc, tile, mybir
from concourse.ap import AP
from concourse.bass_utils import run_bass_kernel_spmd

N, M, D, H, DQK, DV = 2048, 2048, 1024, 16, 64, 64
RP = 2048
NCORES = 8
NB = N // 128
KT = D // 128
F2 = RP + 128
SLICE = N // NCORES

BF = mybir.dt.bfloat16
FP16 = mybir.dt.float16
F32 = mybir.dt.float32
RG = [list(range(NCORES))]
_cache = {}


def _build():
    nc = bacc.Bacc("TRN2", target_bir_lowering=False, debug=False,
                   num_devices=NCORES)
    ACTF = mybir.ActivationFunctionType
    ADD = mybir.AluOpType.add
    MULT = mybir.AluOpType.mult

    xqT = nc.dram_tensor("xqT", [D, N], BF, kind="ExternalInput")
    xkvT = nc.dram_tensor("xkvT", [D, M], BF, kind="ExternalInput")
    scT = nc.dram_tensor("scT", [D, RP], BF, kind="ExternalInput")
    wqT = nc.dram_tensor("wqT", [128, KT * 128], BF, kind="ExternalInput")
    wkT = nc.dram_tensor("wkT", [128, KT * 128], BF, kind="ExternalInput")
    wvT = nc.dram_tensor("wvT", [128, KT * 128], BF, kind="ExternalInput")
    fpeT = nc.dram_tensor("fpeT", [128, KT * 128], BF, kind="ExternalInput")
    woT = nc.dram_tensor("woT", [128, KT * D], BF, kind="ExternalInput")
    identc = nc.dram_tensor("identc", [128, 128], FP16, kind="ExternalInput")
    out_ext = nc.dram_tensor("out", [SLICE, D], F32, kind="ExternalOutput")

    cc_in = [nc.dram_tensor(f"cc_in{h}", [128 * NCORES, 128], BF)
             for h in range(2)]
    cc_out = [nc.dram_tensor(f"cc_out{h}", [128 * NCORES, 128], BF)
              for h in range(2)]

    with tile.TileContext(nc) as tc:
        with (
            tc.tile_pool(name="const", bufs=1) as cpool,
            tc.tile_pool(name="proj", bufs=1) as proj,
            tc.tile_pool(name="xq_s", bufs=2) as xq_s,
            tc.tile_pool(name="xkv_s", bufs=2) as xkv_s,
            tc.tile_pool(name="sc_s", bufs=2) as sc_s,
            tc.tile_pool(name="work", bufs=2) as work,
            tc.tile_pool(name="small", bufs=2) as small,
            tc.tile_pool(name="psA", bufs=6, space="PSUM") as psA,
            tc.tile_pool(name="psC", bufs=2, space="PSUM") as psC,
        ):
            # ---- weights (pre-shuffled on host for contiguous DMA)
            wq_sb = cpool.tile([128, KT, 128], BF, tag="wq")
            wk_sb = cpool.tile([128, KT, 128], BF, tag="wk")
            wv_sb = cpool.tile([128, KT, 128], BF, tag="wv")
            fpe_sb = cpool.tile([128, KT, 128], BF, tag="fpe")
            for dst, srcw in ((wq_sb, wqT), (wk_sb, wkT), (wv_sb, wvT),
                              (fpe_sb, fpeT)):
                nc.gpsimd.dma_start(
                    dst[:], srcw.ap().rearrange("p (k c) -> p k c", k=KT))
            wo_sb = cpool.tile([128, KT, D], BF, tag="wo")
            ident = cpool.tile([128, 128], FP16, tag="ident")
            nc.gpsimd.dma_start(ident[:], identc[:])

            # ---- persistent activations
            q2T = proj.tile([128, N], BF, tag="q2T")
            k2T = proj.tile([128, M], BF, tag="k2T")
            pe2T = proj.tile([128, RP], BF, tag="pe2T")
            # V in [m, dv] tiles with a ones column (row-sum trick)
            v2e = [proj.tile([128, NB, 65], BF, tag=f"v2e{h}",
                             name=f"v2e{h}") for h in range(2)]
            for h in range(2):
                nc.gpsimd.memset(v2e[h][:, :, 64:65], 1.0)
            # P^T slot buffer: 8 slots of n-tiles (double-buffered chunks)
            PT = [proj.tile([128, NB, 8, 128], BF, tag=f"PT{h}",
                            name=f"PT{h}") for h in range(2)]
            ctxh = [proj.tile([64, N], BF, tag=f"ctxh{h}", name=f"ctxh{h}")
                    for h in range(2)]
            plrbuf = [proj.tile([128, F2], FP16, tag=f"plr{i}",
                                name=f"plr{i}") for i in range(4)]
            for i in range(4):
                nc.gpsimd.memset(plrbuf[i][:, RP:F2], -60000.0)

            # round-robin copy engines for psum->sbuf copies
            cp = [0]

            def copy(dst, srcp):
                eng = cp[0] & 1
                cp[0] += 1
                if eng == 0:
                    nc.scalar.activation(dst, srcp, ACTF.Copy)
                else:
                    nc.vector.tensor_copy(dst, srcp)

            # ---- emission helpers ------------------------------------
            def load_xq(ch):
                t = xq_s.tile([128, KT, 512], BF, tag="xq", name=f"xq{ch}")
                src = AP(xqT, ch * 512,
                         [[N, 128], [128 * N, KT], [1, 512]])
                nc.sync.dma_start(t[:], src)
                return t

            def load_xkv(ch):
                t = xkv_s.tile([128, KT, 512], BF, tag="xkv",
                               name=f"xkv{ch}")
                src = AP(xkvT, ch * 512,
                         [[M, 128], [128 * M, KT], [1, 512]])
                nc.gpsimd.dma_start(t[:], src)
                return t

            def load_sc(ch):
                t = sc_s.tile([128, KT, 512], BF, tag="sc", name=f"sc{ch}")
                src = AP(scT, ch * 512,
                         [[RP, 128], [128 * RP, KT], [1, 512]])
                nc.gpsimd.dma_start(t[:], src)
                return t

            def proj_chunk(wtile, xt, dest_col, dest, name):
                ps = psA.tile([128, 512], F32, tag="psA", name=f"pj{name}")
                for k in range(KT):
                    nc.tensor.matmul(ps[:], wtile[:, k, :], xt[:, k, :],
                                     start=(k == 0), stop=(k == KT - 1))
                copy(dest[:, dest_col:dest_col + 512], ps[:])

            def proj_v(xt, ch):
                ps = psA.tile([128, 512], F32, tag="psA", name=f"pjv{ch}")
                for k in range(KT):
                    nc.tensor.matmul(ps[:], wv_sb[:, k, :], xt[:, k, :],
                                     start=(k == 0), stop=(k == KT - 1))
                v2t = work.tile([128, 512], BF, tag="v2t", bufs=1,
                                name=f"v2t{ch}")
                copy(v2t[:], ps[:])
                # full-128-partition xbar transpose (64-part inputs break on
                # HW), then split the two heads with strided engine copies
                v2a = work.tile([128, 4, 128], BF, tag="v2a", bufs=1,
                                name=f"v2a{ch}")
                nc.sync.dma_start_transpose(v2a[:], v2t[:])
                for hl in range(2):
                    hb = hl * 64
                    copy(v2e[hl][:, ch * 4:(ch + 1) * 4, 0:64],
                         v2a[:, :, hb:hb + 64])

            def block(nb, skew_eng=None):
                skew_eng = skew_eng or nc.gpsimd
                n0 = nb * 128
                span = n0 + 128
                rlo = RP - 1 - n0 - 127  # lowest rel-position read by skew
                c_lo = rlo // 512
                nch = (span + 511) // 512
                for hl in range(2):
                    hb = hl * 64
                    plr = plrbuf[(2 * nb + hl) % 4]
                    # position logits, narrowed to the needed window
                    for ch in range(c_lo, RP // 512):
                        lo = max(ch * 512, rlo)
                        w = (ch + 1) * 512 - lo
                        ps = psA.tile([128, 512], F32, tag="psA",
                                      name=f"plr{nb}_{hl}_{ch}")
                        nc.tensor.matmul(
                            ps[:, :w], q2T[hb:hb + 64, n0:n0 + 128],
                            pe2T[hb:hb + 64, lo:lo + w],
                            start=True, stop=True)
                        copy(plr[:, lo:lo + w], ps[:, :w])
                    # skew gather: diagonal read
                    sS = work.tile([128, N], FP16, tag="sS",
                                   name=f"sS{nb}_{hl}")
                    s0 = plr[:]
                    base = s0.offset + (RP - 1 - n0)
                    skew_eng.dma_start(
                        sS[:, 0:span],
                        AP(s0.tensor, base, [[F2 - 1, 128], [1, span]]))
                    # content logits; pos added via identity matmul
                    cps = []
                    for ch in range(nch):
                        cw = min(512, span - ch * 512)
                        ps = psA.tile([128, 512], F32, tag="psA",
                                      name=f"cont{nb}_{hl}_{ch}")
                        nc.tensor.matmul(
                            ps[:, :cw], q2T[hb:hb + 64, n0:n0 + 128],
                            k2T[hb:hb + 64, ch * 512:ch * 512 + cw],
                            start=True, stop=False)
                        cps.append((ps, cw))
                    for ch, (ps, cw) in enumerate(cps):
                        nc.tensor.matmul(
                            ps[:, :cw], ident[:],
                            sS[:, ch * 512:ch * 512 + cw],
                            start=False, stop=True)
                    pP = work.tile([128, N], BF, tag="pP",
                                   name=f"pP{nb}_{hl}")
                    for ch, (ps, cw) in enumerate(cps):
                        nc.scalar.activation(
                            pP[:, ch * 512:ch * 512 + cw], ps[:, :cw],
                            ACTF.Exp)
                    nc.sync.dma_start_transpose(
                        PT[hl][:, 0:nb + 1, nb % 8, :], pP[:, 0:span])

            ctx_live = {}

            def ctx_mm(c, part):
                # part: (col_lo, col_hi) within the 512-wide chunk
                col_lo, col_hi = part
                sbase = (c % 2) * 4
                slo, shi = sbase + col_lo // 128, sbase + col_hi // 128
                for hl in range(2):
                    ps = psC.tile([65, 512], F32, tag="psC",
                                  name=f"ctx{c}_{hl}_{col_lo}")
                    mt_hi = 4 * c + col_hi // 128 - 1
                    first = True
                    for mt in range(mt_hi + 1):
                        s0 = max(slo, sbase + (mt - 4 * c))
                        if s0 >= shi:
                            continue
                        p0 = (s0 - sbase) * 128
                        nc.tensor.matmul(
                            ps[:, p0:col_hi],
                            v2e[hl][:, mt, :],
                            PT[hl][:, mt, s0:shi, :],
                            start=first, stop=(mt == mt_hi),
                            skip_group_check=True)
                        first = False
                    # row 64 is the softmax denominator; invert it now
                    rc = small.tile([1, 512], F32, tag=f"rc{hl}",
                                    bufs=1, name=f"rc{c}_{hl}_{col_lo}")
                    with nc.allow_low_precision(reason="1/l for softmax"):
                        nc.vector.reciprocal(rc[:, col_lo:col_hi],
                                             ps[64:65, col_lo:col_hi])
                    ctx_live[(c, col_lo, hl)] = (ps, rc)

            def ctx_norm(c, part):
                # emitted a couple of blocks after ctx_mm so the gpsimd
                # queue reaches the broadcast with its input long ready
                col_lo, col_hi = part
                for hl in range(2):
                    ps, rc = ctx_live.pop((c, col_lo, hl))
                    bc = small.tile([64, 512], F32, tag="bcast",
                                    name=f"bc{c}_{hl}_{col_lo}")
                    nc.gpsimd.partition_broadcast(bc[:, col_lo:col_hi],
                                                  rc[:, col_lo:col_hi])
                    nc.vector.tensor_tensor(
                        ctxh[hl][:, c * 512 + col_lo:c * 512 + col_hi],
                        ps[0:64, col_lo:col_hi], bc[:, col_lo:col_hi], MULT)

            def cc_write(half):
                cc_i = cc_in[half]
                for hl in range(2):
                    dst = AP(cc_i, hl * 64 * 128,
                             [[128, 64], [128 * 128, NCORES], [1, 128]])
                    s = ctxh[hl][:]
                    src = AP(s.tensor, s.offset + half * 1024,
                             [[N, 64], [128, NCORES], [1, 128]])
                    nc.sync.dma_start(dst, src)

            def cc_fire(half):
                nc.gpsimd.collective_compute(
                    "AllToAll", mybir.AluOpType.bypass,
                    ins=[cc_in[half][:]], outs=[cc_out[half][:]],
                    replica_groups=RG)

            def outproj(half):
                stg = small.tile([128, KT, 128], BF, tag="stg",
                                 bufs=1, name=f"stg{half}")
                nc.gpsimd.dma_start(
                    stg[:], AP(cc_out[half], 0,
                               [[128, 128], [128 * 128, KT], [1, 128]]))
                for dc in range(D // 512):
                    ps = psA.tile([128, 512], F32, tag="psA",
                                  name=f"out{half}_{dc}")
                    for k in range(KT):
                        nc.tensor.matmul(
                            ps[:], stg[:, k, :],
                            wo_sb[:, k, dc * 512:(dc + 1) * 512],
                            start=(k == 0), stop=(k == KT - 1))
                    ostage = small.tile([128, 512], F32, tag="ostage",
                                        bufs=1, name=f"ost{half}_{dc}")
                    nc.scalar.activation(ostage[:], ps[:], ACTF.Copy)
                    nc.sync.dma_start(
                        out_ext[half * 128:(half + 1) * 128,
                                dc * 512:(dc + 1) * 512], ostage[:])

            # ---- emission schedule -----------------------------------
            xq = [None] * 4
            xkv = [None] * 4
            sc = [None] * 4
            xq[0] = load_xq(0)
            xkv[0] = load_xkv(0)
            sc[3] = load_sc(3)

            proj_chunk(wq_sb, xq[0][:], 0, q2T, "q0")
            proj_chunk(wk_sb, xkv[0][:], 0, k2T, "k0")
            proj_chunk(fpe_sb, sc[3][:], 1536, pe2T, "pe3")

            xq[1] = load_xq(1)
            xkv[1] = load_xkv(1)
            sc[2] = load_sc(2)

            block(0)
            block(1)

            proj_chunk(wq_sb, xq[1][:], 512, q2T, "q1")
            proj_chunk(wk_sb, xkv[1][:], 512, k2T, "k1")
            proj_v(xkv[0][:], 0)
            proj_chunk(fpe_sb, sc[2][:], 1024, pe2T, "pe2")

            xq[2] = load_xq(2)
            xkv[2] = load_xkv(2)
            sc[1] = load_sc(1)

            block(2)
            nc.gpsimd.dma_start(
                wo_sb[:], woT.ap().rearrange("p (k c) -> p k c", k=KT))
            block(3)

            proj_chunk(wq_sb, xq[2][:], 1024, q2T, "q2")
            proj_chunk(wk_sb, xkv[2][:], 1024, k2T, "k2")
            proj_v(xkv[1][:], 1)
            proj_chunk(fpe_sb, sc[1][:], 512, pe2T, "pe1")

            xq[3] = load_xq(3)
            xkv[3] = load_xkv(3)
            sc[0] = load_sc(0)

            block(4)
            ctx_mm(0, (0, 512))
            block(5)

            proj_chunk(wq_sb, xq[3][:], 1536, q2T, "q3")
            proj_chunk(wk_sb, xkv[3][:], 1536, k2T, "k3")
            proj_v(xkv[2][:], 2)
            proj_chunk(fpe_sb, sc[0][:], 0, pe2T, "pe0")

            ctx_norm(0, (0, 512))
            block(6)
            block(7)
            proj_v(xkv[3][:], 3)
            block(8)
            ctx_mm(1, (0, 512))
            block(9)
            ctx_norm(1, (0, 512))
            cc_write(0)
            block(10)
            block(11)
            cc_fire(0)
            block(12, skew_eng=nc.scalar)
            ctx_mm(2, (0, 512))
            block(13, skew_eng=nc.scalar)
            ctx_norm(2, (0, 512))
            block(14, skew_eng=nc.scalar)
            ctx_mm(3, (0, 256))
            outproj(0)
            ctx_norm(3, (0, 256))
            block(15, skew_eng=nc.scalar)
            ctx_mm(3, (256, 512))
            ctx_norm(3, (256, 512))
            cc_write(1)
            cc_fire(1)
            outproj(1)

    nc.compile()
    return nc


def _host_prep(inputs):
    bf16 = ml_dtypes.bfloat16
    x_q = np.asarray(inputs["x_q"])[:, 0, :]
    x_kv = np.asarray(inputs["x_kv"])[:, 0, :]
    to_q = np.asarray(inputs["to_q"])
    to_k = np.asarray(inputs["to_k"])
    to_v = np.asarray(inputs["to_v"])
    to_out = np.asarray(inputs["to_out"])
    fpe = np.asarray(inputs["for_pos_enc"])

    xqT = np.ascontiguousarray(x_q.T).astype(bf16)
    xkvT = np.ascontiguousarray(x_kv.T).astype(bf16)

    r = np.arange(0, RP, dtype=np.float32)
    inv_freq = 1.0 / (10000.0 ** (np.arange(0.0, D, 2.0, np.float32) / D))
    ph = r[:, None] * inv_freq[None, :]
    sincos = np.concatenate([np.sin(ph), np.cos(ph)], axis=-1)
    scT = np.ascontiguousarray(sincos[::-1].T).astype(bf16)

    wo_ckd = (to_out.transpose(0, 2, 1).reshape(D, H * DV).T
              .reshape(KT, 128, D).transpose(1, 0, 2).reshape(128, KT * D))
    woT = np.ascontiguousarray(wo_ckd).astype(bf16)
    identity = np.eye(128, dtype=np.float16)

    def shuf(w):
        # [D, 128] -> [p, k*c] with the k-tile index on the free axis
        return np.ascontiguousarray(
            w.reshape(KT, 128, 128).transpose(1, 0, 2).reshape(128, KT * 128)
        ).astype(bf16)

    in_maps = []
    for c in range(NCORES):
        hs = [2 * c, 2 * c + 1]
        in_maps.append({
            "xqT": xqT, "xkvT": xkvT, "scT": scT,
            "wqT": shuf(np.concatenate([to_q[:, h, :].T for h in hs], 1)),
            "wkT": shuf(np.concatenate([to_k[:, h, :].T for h in hs], 1)),
            "wvT": shuf(np.concatenate([to_v[:, h, :].T for h in hs], 1)),
            "fpeT": shuf(np.concatenate([fpe[:, h, :].T for h in hs], 1)),
            "woT": woT, "identc": identity,
        })
    return in_maps


def _assemble(res):
    # core c computed out rows [128c, 128c+128) and [1024+128c, ...)
    out = np.empty((N, D), np.float32)
    for c in range(NCORES):
        r = res.results[c]["out"]
        out[128 * c:128 * (c + 1)] = r[0:128]
        out[1024 + 128 * c:1024 + 128 * (c + 1)] = r[128:256]
    return out.reshape(N, 1, D)


def kernel(**inputs):
    if "nc" not in _cache:
        _cache["nc"] = _build()
    nc = _cache["nc"]
    in_maps = _host_prep(inputs)
    res = run_bass_kernel_spmd(nc, in_maps, list(range(NCORES)))
    return _assemble(res).astype(np.float32)


if __name__ == "__main__":
    import pickle
    with open("/tmp/inputs.pkl", "rb") as f:
        inputs = pickle.load(f)
    out = kernel(**inputs)
    exp = np.load("/tmp/expected.npy")
    err = np.linalg.norm(out - exp) / np.linalg.norm(exp)
    print("Relative error:", err)
